# revision 1
# baseline (speedup 1.0000x reference)
"""Trainium2 Bass kernel for nn_DPSpikingDecoder.

Math: the leaky-integrator scan v_t = 0.5*v_{t-1} + x_t, the mean over
channels C, and the differential window pooling are all linear maps over
the time axis, and the scan kernel is identical for every channel.  So

    dp[b, w, f] = sum_{c,t} (K[w, t] / C) * spikes[b, c, t, f]

where K = M_pool @ L_scan is a [W=40, T=960] matrix precomputed on host.
Viewing spikes[b] as a flat [C*T, F] matrix, this is one 30720-long
matmul contraction per sample, streamed through the PE in 240 chunks of
128 rows while spikes stream from HBM exactly once (memory-bound, runs
at the ~350 GB/s per-core HBM roofline).  The weight tile for chunk m
depends only on m mod 15 (lcm(128, 960) = 1920 = 15*128), so 15 weight
tiles stay resident in SBUF.  float32r gives the full-rate PE path
(1 cycle/row at N=256) on unmodified fp32 bytes.

The tiny MLP + softmax + scale run on-chip as a short tail; layer 1 is
packed 4-wide into PE column groups via tile_position.

Sharding: data-parallel over batch B=8 -> one sample per NeuronCore.
"""

import numpy as np
from contextlib import ExitStack

import concourse.bass as bass
import concourse.bacc as bacc
import concourse.tile as tile
from concourse import mybir
from concourse.bass_utils import run_bass_kernel_spmd

F32 = mybir.dt.float32
F32R = mybir.dt.float32r

B, C, T, F = 8, 32, 960, 256
L_DP, N_DP = 24, 12
W = T // L_DP            # 40 windows
H = 20                   # hidden dim of the MLP

R = C * T                # 30720 contraction rows per sample
CH = 128                 # rows per matmul chunk
NCH = R // CH            # 240 chunks
QP = 15                  # weight-tile period: lcm(128, 960) / 128
CPD = 12                 # chunks per xt tile (1.5 MB, loaded as 2 half-DMAs)
ND = NCH // CPD          # 20 streaming tiles
HB = CPD // 2            # chunks per half-DMA


def _host_K():
    """K[w, t] in float64: differential pooling of the decayed scan."""
    t = np.arange(T)
    d = t[:, None] - t[None, :]
    Lmat = np.where(d >= 0, 0.5 ** np.clip(d, 0, None), 0.0)
    M = np.zeros((W, T))
    for w in range(W):
        M[w, w * L_DP + L_DP - N_DP : w * L_DP + L_DP] = 1.0 / N_DP
        M[w, w * L_DP : w * L_DP + N_DP] -= 1.0 / N_DP
    return M @ Lmat  # [W, T]


def _host_kt():
    """SBUF image [CH, QP*W]: kt[p, q*W+w] = K[w, (128q+p)%960]/C."""
    K = _host_K()
    q = np.arange(QP)[:, None]
    p = np.arange(CH)[None, :]
    tidx = (CH * q + p) % T                      # [QP, CH]
    kt2 = K.T[tidx] / C                          # [QP, CH, W]
    img = kt2.transpose(1, 0, 2).reshape(CH, QP * W)
    return np.ascontiguousarray(img.astype(np.float32))


def _host_cimg(W2, b2):
    """Packed small consts, one contiguous [128, 101] DMA image:
    cols 0:40 eye(40) on parts 0:40; 40:80 [W2; b2] on parts 0:21;
    col 80 b1 placeholder (zeros, real b1 patched in kernel());
    cols 81:101 the 4-col-group summing matrix."""
    img = np.zeros((128, 101), dtype=np.float32)
    img[0:W, 0:W] = np.eye(W, dtype=np.float32)
    img[0:H, 40:80] = W2.astype(np.float32)
    img[H, 40:80] = b2.astype(np.float32)
    for j in range(4):
        for i in range(H):
            img[32 * j + i, 81 + i] = 1.0
    return img


def _build_program():
    nc = bacc.Bacc(None)
    x = nc.declare_dram_parameter("x", [ND, CPD * CH, F], F32R, isOutput=False)
    kt = nc.declare_dram_parameter("kt", [CH, QP * W], F32R, isOutput=False)
    w1r = nc.declare_dram_parameter("w1r", [128, 2 * W * H], F32, isOutput=False)
    cimg = nc.declare_dram_parameter("cimg", [128, 101], F32, isOutput=False)
    y = nc.declare_dram_parameter("y", [W, F], F32, isOutput=True)

    with tile.TileContext(nc) as tc, ExitStack() as ctx:
        consts = ctx.enter_context(tc.tile_pool(name="consts", bufs=1))
        xs = ctx.enter_context(tc.tile_pool(name="xs", bufs=10))
        work = ctx.enter_context(tc.tile_pool(name="work", bufs=1))
        dp_psp = ctx.enter_context(tc.tile_pool(name="dp_ps", bufs=1, space="PSUM"))
        sm_ps = ctx.enter_context(tc.tile_pool(name="sm_ps", bufs=1, space="PSUM"))

        # kt first on the sync HWDGE ring (the PE needs it for the first MM);
        # both const images are contiguous per-partition, so the DMAs are fast.
        kt_sb = consts.tile([CH, QP, W], F32R)
        nc.sync.dma_start(out=kt_sb, in_=kt[:].rearrange("p (q w) -> p q w", q=QP))
        ci_sb = consts.tile([128, 101], F32)
        nc.scalar.dma_start(out=ci_sb, in_=cimg[:])
        eye_sb = ci_sb[0:W, 0:W]
        w2b_sb = ci_sb[0 : H + 1, 40:80]
        b1_sb = ci_sb[0:H, 80:81]
        sel_sb = ci_sb[:, 81:101]
        # w1 is tail-only; its DMA is emitted AFTER the x stream so it rides
        # at the end of the sync ring and doesn't steal ramp bandwidth.
        w1_sb = consts.tile([128, 2 * W * H], F32)

        # augmented MLP input [h; 1] so layer 2 adds b2 inside the matmul
        h_aug = work.tile([H + 1, 1], F32)
        nc.vector.memset(h_aug, 1.0)  # row H stays 1; rows 0..H-1 overwritten

        # ---- big streamed contraction: dp[w, f] += kt_q^T @ x_chunk ----
        dp_ps = dp_psp.tile([W, F], F32)
        for d in range(ND - 1):
            xt_a = xs.tile([CH, HB, F], F32R)
            xt_b = xs.tile([CH, HB, F], F32R)
            for eng, xt, h2 in ((nc.sync, xt_a, 0), (nc.scalar, xt_b, 1)):
                eng.dma_start(
                    out=xt,
                    in_=x[d, h2 * HB * CH : (h2 + 1) * HB * CH].rearrange(
                        "(s p) f -> p s f", p=CH
                    ),
                )
            for s in range(CPD):
                m = d * CPD + s
                xt = xt_a if s < HB else xt_b
                nc.tensor.matmul(
                    dp_ps,
                    lhsT=kt_sb[:, m % QP, :],
                    rhs=xt[:, s % HB, :],
                    start=(m == 0),
                    stop=False,
                )
        # last tile arrives as four quarter-DMAs so the final matmuls can
        # drain as soon as each 3-chunk slice lands
        d = ND - 1
        QB = CPD // 4
        for qd in range(4):
            xt_q = xs.tile([CH, QB, F], F32R, tag="xt_q", bufs=4)
            eng = nc.sync if qd % 2 == 0 else nc.scalar
            eng.dma_start(
                out=xt_q,
                in_=x[d, qd * QB * CH : (qd + 1) * QB * CH].rearrange(
                    "(s p) f -> p s f", p=CH
                ),
            )
            for s2 in range(QB):
                m = d * CPD + qd * QB + s2
                nc.tensor.matmul(
                    dp_ps,
                    lhsT=kt_sb[:, m % QP, :],
                    rhs=xt_q[:, s2, :],
                    start=False,
                    stop=(m == NCH - 1),
                )

        # w1 rides at the very end of both rings, split so neither ring's
        # x stream is delayed and the halves land concurrently
        nc.sync.dma_start(out=w1_sb[:, 0 : W * H], in_=w1r[:, 0 : W * H])
        nc.scalar.dma_start(out=w1_sb[:, W * H :], in_=w1r[:, W * H :])

        dp_sb = work.tile([W, F], F32)
        nc.vector.tensor_copy(dp_sb, dp_ps)

        # ---- transpose dp to feed the MLP contraction ----
        dpT_ps = sm_ps.tile([128, 2, W], F32)
        for e in range(2):
            nc.tensor.transpose(dpT_ps[:, e, :], dp_sb[:, e * 128 : (e + 1) * 128], eye_sb)
        dpT_sb = work.tile([128, 2, W], F32)
        nc.vector.tensor_copy(dpT_sb, dpT_ps)

        # ---- layer 1: h = relu(dp_flat @ W1 + b1), 80 chunks of 128 ----
        # packed 4-wide into PE column groups; partial sums land in four
        # partition slices of hp_ps and are summed by one sel-matmul.
        hp_ps = sm_ps.tile([128, 1], F32)
        for m in range(2 * W):
            w, e = divmod(m, 2)
            j = m % 4
            nc.tensor.matmul(
                hp_ps[32 * j : 32 * j + H, :],
                lhsT=w1_sb[:, m * H : (m + 1) * H],
                rhs=dpT_sb[:, e, w : w + 1],
                start=(m < 4),
                stop=(m >= 2 * W - 4),
                tile_position=(0, 32 * j),
            )
        hp_sb = work.tile([128, 1], F32)
        nc.vector.tensor_copy(hp_sb, hp_ps)
        h_ps = sm_ps.tile([H, 1], F32)
        nc.tensor.matmul(h_ps, lhsT=sel_sb, rhs=hp_sb, start=True, stop=True)
        nc.scalar.activation(
            h_aug[0:H, :], h_ps, mybir.ActivationFunctionType.Relu, bias=b1_sb
        )

        # ---- layer 2 (+b2 via augmented row) + softmax on a [1, W] row ----
        a2_ps = sm_ps.tile([1, W], F32)
        nc.tensor.matmul(a2_ps, lhsT=h_aug, rhs=w2b_sb, start=True, stop=True)
        e_sb = work.tile([1, W], F32)
        ssum = work.tile([1, 1], F32)
        nc.scalar.activation(
            e_sb, a2_ps, mybir.ActivationFunctionType.Exp, accum_out=ssum[:]
        )
        rin = work.tile([1, 1], F32)
        nc.vector.reciprocal(rin, ssum)
        ta_sb = work.tile([1, W], F32)
        nc.vector.tensor_scalar_mul(ta_sb, e_sb, rin[:])

        # ---- scale dp rows by attention weights and store ----
        taT_ps = sm_ps.tile([W, 1], F32)
        nc.tensor.transpose(taT_ps, ta_sb, ci_sb[0:1, 0:1])
        ta_col = work.tile([W, 1], F32)
        nc.vector.tensor_copy(ta_col, taT_ps)
        att = work.tile([W, F], F32)
        for eng, e2 in ((nc.sync, 0), (nc.scalar, 1)):
            nc.vector.tensor_scalar_mul(
                att[:, e2 * 128 : (e2 + 1) * 128],
                dp_sb[:, e2 * 128 : (e2 + 1) * 128],
                ta_col[:],
            )
            eng.dma_start(
                out=y[:, e2 * 128 : (e2 + 1) * 128],
                in_=att[:, e2 * 128 : (e2 + 1) * 128],
            )

    nc.compile()
    return nc


_CACHED = {}


def _get_program():
    if "nc" not in _CACHED:
        _CACHED["nc"] = _build_program()
        _CACHED["kt"] = _host_kt()
    return _CACHED["nc"]


def _in_maps(spikes, W1, b1, W2, b2):
    spikes = np.ascontiguousarray(np.asarray(spikes, dtype=np.float32))
    W1 = np.asarray(W1, dtype=np.float32)
    b1 = np.asarray(b1, dtype=np.float32)
    W2 = np.asarray(W2, dtype=np.float32)
    b2 = np.asarray(b2, dtype=np.float32)
    _get_program()
    # W1 rearranged so chunk m = 2*w + e holds rows d = 256*w + 128*e + p,
    # laid out so the DMA is one contiguous [128, 1600] block.
    w1r = np.ascontiguousarray(
        W1.reshape(W, 2, 128, H).transpose(2, 0, 1, 3).reshape(128, 2 * W * H)
    )
    cimg = _host_cimg(W2, b2)
    cimg[0:H, 80] = b1
    shared = {"kt": _CACHED["kt"], "w1r": w1r, "cimg": cimg}
    return [
        {"x": spikes[b].reshape(ND, CPD * CH, F), **shared}
        for b in range(B)
    ]


def kernel(spikes, W1, b1, W2, b2):
    in_maps = _in_maps(spikes, W1, b1, W2, b2)
    res = run_bass_kernel_spmd(_get_program(), in_maps, list(range(B)))
    out = np.stack([np.asarray(res.results[i]["y"]).reshape(W * F) for i in range(B)])
    return out.astype(np.float32)



# revision 8
# speedup vs baseline: 1.6087x; 1.6087x over previous
"""Trainium2 Bass kernel for nn_DPSpikingDecoder.

Math: the leaky-integrator scan v_t = 0.5*v_{t-1} + x_t, the channel mean,
and the differential window pooling compose into one linear kernel over
time:  dp[b, w, f] = sum_{c,t} (K[w, t] / C) * spikes[b, c, t, f].
K is banded and window-periodic: window w sees its own 24 steps (kernel
Kd) plus the previous window's 24 steps through the 0.5^d decay tail
(kernel Kt); deeper history is < 1e-7 relative and dropped.

The stream is laid out t-major on host (row r = t*C + c) so each 128-row
chunk is 4 timesteps x 32 channels and 6 chunks complete one window.
Each chunk's matmul uses an M=33 stationary [Kd, 0*31, Kt] so the PSUM
tile collects A_w at partition 0 and the B-tail at partition 32 — both
legal bases for engine reads (partition bases are quantized to 32).
dp[w] = A_w + B_{w-1} is one DVE add (one PSUM + one SBUF operand), and
rows are scattered to their partitions via SBUF->SBUF DMAs on the idle
gpsimd (SWDGE) ring so the HWDGE x-stream rings never block.

Windows finalize every ~1.1us while the stream runs; the MLP head
(batched PE transposes at w=31/38, 80 packed layer-1 matmuls) overlaps
the stream, leaving only a short scalar chain after the last byte.

x is uploaded as float16 (PSUM accumulates fp32): halves the HBM
traffic, which is the roofline (~350 GB/s/core measured); end-to-end
rel err ~4e-4 vs the fp32 reference (gate 2e-2).  The softmax
denominator ships to host, which divides during unsharding.

Sharding: data-parallel over batch B=8 -> one sample per NeuronCore.
"""

import numpy as np
from contextlib import ExitStack

import concourse.bass as bass
import concourse.bacc as bacc
import concourse.tile as tile
from concourse import mybir
from concourse.bass_utils import run_bass_kernel_spmd

F16 = mybir.dt.float16
F32 = mybir.dt.float32

B, C, T, F = 8, 32, 960, 256
L_DP, N_DP = 24, 12
W = T // L_DP            # 40 windows
H = 20                   # hidden dim of the MLP
CH = 128                 # rows per matmul chunk
S6 = 6                   # chunks per window (6 * 128 rows = 24 t * 32 c)
MS = 33                  # stationary width: Kd at col 0, Kt at col 32


def _host_K():
    """Exact K[w, t] = differential pooling of the decayed scan."""
    t = np.arange(T)
    d = t[:, None] - t[None, :]
    Lmat = np.where(d >= 0, 0.5 ** np.clip(d, 0, None), 0.0)
    M = np.zeros((W, T))
    for w in range(W):
        M[w, w * L_DP + L_DP - N_DP : w * L_DP + L_DP] = 1.0 / N_DP
        M[w, w * L_DP : w * L_DP + N_DP] -= 1.0 / N_DP
    return M @ Lmat  # [W, T]


def _host_kt_img():
    """[128, 256] fp16 image; chunk s owns cols [33s, 33s+33): col 33s = Kd,
    col 33s+32 = Kt, zeros between.  Row p has t-offset u = 4s + p//32."""
    K = _host_K()
    Kd = K[1, 24:48] / C   # within-window kernel (w-independent, verified)
    Kt = K[2, 24:48] / C   # decay tail onto the next window
    img = np.zeros((CH, 256), dtype=np.float32)
    u = 4 * (np.arange(S6)[None, :]) + (np.arange(CH) // 32)[:, None]  # [128, 6]
    for s in range(S6):
        img[:, MS * s] = Kd[u[:, s]]
        img[:, MS * s + 32] = Kt[u[:, s]]
    return img.astype(np.float16)


def _host_cimg(W2, b2):
    """Packed fp32 consts, one [128, 128] DMA image: cols 0:40 eye(40);
    40:80 [W2; b2]; col 80 b1 (patched in _in_maps); 81:101 the 4-col-group
    summing matrix; col 101 ones (softmax-denominator sum vector)."""
    img = np.zeros((CH, 128), dtype=np.float32)
    img[0:W, 0:W] = np.eye(W, dtype=np.float32)
    img[0:H, 40:80] = W2.astype(np.float32)
    img[H, 40:80] = b2.astype(np.float32)
    for j in range(4):
        for i in range(H):
            img[32 * j + i, 81 + i] = 1.0
    img[0:W, 101] = 1.0
    return img


def _build_program():
    nc = bacc.Bacc(None)
    x = nc.declare_dram_parameter("x", [W, CH, S6, F], F16, isOutput=False)
    ktp = nc.declare_dram_parameter("ktp", [CH, 256], F16, isOutput=False)
    w1r = nc.declare_dram_parameter("w1r", [CH, 2 * W * H], F16, isOutput=False)
    cimg = nc.declare_dram_parameter("cimg", [CH, 128], F32, isOutput=False)
    y = nc.declare_dram_parameter("y", [W, F], F32, isOutput=True)
    sden = nc.declare_dram_parameter("sden", [1, 1], F32, isOutput=True)

    with tile.TileContext(nc) as tc, ExitStack() as ctx:
        consts = ctx.enter_context(tc.tile_pool(name="consts", bufs=1))
        xs = ctx.enter_context(tc.tile_pool(name="xs", bufs=10))
        qbp = ctx.enter_context(tc.tile_pool(name="qbp", bufs=2))
        scp = ctx.enter_context(tc.tile_pool(name="scp", bufs=4))
        dpt = ctx.enter_context(tc.tile_pool(name="dpt", bufs=2))
        work = ctx.enter_context(tc.tile_pool(name="work", bufs=1))
        pp = ctx.enter_context(tc.tile_pool(name="pp", bufs=3, space="PSUM"))
        tp_psp = ctx.enter_context(tc.tile_pool(name="tp_ps", bufs=1, space="PSUM"))
        hp_psp = ctx.enter_context(tc.tile_pool(name="hp_ps", bufs=1, space="PSUM"))
        tl_psp = ctx.enter_context(tc.tile_pool(name="tl_ps", bufs=2, space="PSUM"))

        kt_sb = consts.tile([CH, 256], F16)
        nc.sync.dma_start(out=kt_sb, in_=ktp[:])
        ci_sb = consts.tile([CH, 128], F32)
        nc.scalar.dma_start(out=ci_sb, in_=cimg[:])
        eye_sb = ci_sb[0:W, 0:W]
        w2b_sb = ci_sb[0 : H + 1, 40:80]
        b1_sb = ci_sb[0:H, 80:81]
        sel_sb = ci_sb[:, 81:101]
        ones_col = ci_sb[0:W, 101:102]
        w1_sb = consts.tile([CH, 2 * W * H], F16)

        # augmented MLP input [h; 1] so layer 2 adds b2 inside the matmul
        h_aug = work.tile([H + 1, 1], F32)
        nc.vector.memset(h_aug, 1.0)  # row H stays 1; rows 0..H-1 overwritten
        dp_sb = work.tile([W, F], F32)
        hp_ps = hp_psp.tile([128, 1], F32)

        qb_prev = None
        sc39 = None
        m2 = 0
        for w in range(W):
            eng = nc.sync if w % 2 == 0 else nc.scalar
            xt = xs.tile([CH, S6, F], F16)
            if w < W - 1:
                eng.dma_start(out=xt, in_=x[w])
            else:
                # last window lands as three 2-chunk slices so its matmuls
                # drain while the final bytes stream in
                for sub, e2 in enumerate((nc.sync, nc.scalar, nc.sync)):
                    e2.dma_start(
                        out=xt[:, 2 * sub : 2 * sub + 2, :],
                        in_=x[w, :, 2 * sub : 2 * sub + 2, :],
                    )
            # w1 rides early on both rings, behind the first x tiles
            if w == 2:
                nc.sync.dma_start(out=w1_sb[:, 0 : W * H], in_=w1r[:, 0 : W * H])
            if w == 3:
                nc.scalar.dma_start(out=w1_sb[:, W * H :], in_=w1r[:, W * H :])

            # ---- window contraction: A_w -> partition 0, B_w -> partition 32
            P = pp.tile([MS, F], F32)
            for s in range(S6):
                nc.tensor.matmul(
                    P,
                    lhsT=kt_sb[:, MS * s : MS * s + MS],
                    rhs=xt[:, s, :],
                    start=(s == 0),
                    stop=(s == S6 - 1),
                )
            # dp[w] = A_w + B_{w-1}: bounce B through SBUF (only one PSUM
            # operand allowed per DVE op), combine at base 0
            sc = scp.tile([1, F], F32)
            if w == 0:
                nc.vector.tensor_copy(sc, P[0:1, :])
            else:
                nc.vector.tensor_add(sc, P[0:1, :], qb_prev)
            if w < W - 1:
                qb = qbp.tile([1, F], F32)
                nc.vector.tensor_copy(qb, P[32:33, :])
                qb_prev = qb
            # place the finished row at its partition; SWDGE ring so the
            # HWDGE x-stream queues never wait on compute
            nc.gpsimd.dma_start(out=dp_sb[w : w + 1, :], in_=sc)

            if w == 31:
                tp1 = tp_psp.tile([128, 2, 32], F32, tag="tp")
                for e in range(2):
                    nc.tensor.transpose(
                        tp1[:, e, :],
                        dp_sb[0:32, 128 * e : 128 * (e + 1)],
                        eye_sb[0:32, 0:32],
                    )
                dpT1 = dpt.tile([128, 2, 32], F16, tag="dpT")
                nc.vector.tensor_copy(dpT1, tp1)
                for i2 in range(32):
                    for e in range(2):
                        j = m2 % 4
                        nc.tensor.matmul(
                            hp_ps[32 * j : 32 * j + H, :],
                            lhsT=w1_sb[:, m2 * H : (m2 + 1) * H],
                            rhs=dpT1[:, e, i2 : i2 + 1],
                            start=(m2 < 4),
                            stop=False,
                            tile_position=(0, 32 * j),
                        )
                        m2 += 1
            elif w == 38:
                tp2 = tp_psp.tile([128, 2, 8], F32, tag="tp")
                for e in range(2):
                    nc.tensor.transpose(
                        tp2[:, e, 0:7],
                        dp_sb[32:39, 128 * e : 128 * (e + 1)],
                        eye_sb[32:39, 32:39],
                    )
                dpT2 = dpt.tile([128, 2, 8], F16, tag="dpT")
                nc.vector.tensor_copy(dpT2[:, :, 0:7], tp2[:, :, 0:7])
                for i2 in range(7):
                    for e in range(2):
                        j = m2 % 4
                        nc.tensor.matmul(
                            hp_ps[32 * j : 32 * j + H, :],
                            lhsT=w1_sb[:, m2 * H : (m2 + 1) * H],
                            rhs=dpT2[:, e, i2 : i2 + 1],
                            start=False,
                            stop=(m2 >= 2 * W - 4),
                            tile_position=(0, 32 * j),
                        )
                        m2 += 1
                tp2_keep = tp2
                dpT2_keep = dpT2
            elif w == 39:
                # window 39's row transposes straight from its combine tile
                # (base 0) so nothing waits on the row-placement DMA
                for e in range(2):
                    nc.tensor.transpose(
                        tp2_keep[:, e, 7:8],
                        sc[0:1, 128 * e : 128 * (e + 1)],
                        eye_sb[0:1, 0:1],
                    )
                nc.vector.tensor_copy(dpT2_keep[:, :, 7:8], tp2_keep[:, :, 7:8])
                for e in range(2):
                    j = m2 % 4
                    nc.tensor.matmul(
                        hp_ps[32 * j : 32 * j + H, :],
                        lhsT=w1_sb[:, m2 * H : (m2 + 1) * H],
                        rhs=dpT2_keep[:, e, 7:8],
                        start=False,
                        stop=(m2 >= 2 * W - 4),
                        tile_position=(0, 32 * j),
                    )
                    m2 += 1
                sc39 = sc

        # ---- tail: fold col groups, relu, layer 2 (transposed), exp ----
        hp_sb = work.tile([128, 1], F32)
        nc.vector.tensor_copy(hp_sb, hp_ps)
        h_ps = tl_psp.tile([H, 1], F32, tag="t")
        nc.tensor.matmul(h_ps, lhsT=sel_sb, rhs=hp_sb, start=True, stop=True)
        nc.scalar.activation(
            h_aug[0:H, :], h_ps, mybir.ActivationFunctionType.Relu, bias=b1_sb
        )
        # swapped operands -> logits arrive already transposed [W, 1]
        a2T_ps = tl_psp.tile([W, 1], F32, tag="t")
        nc.tensor.matmul(a2T_ps, lhsT=w2b_sb, rhs=h_aug, start=True, stop=True)
        e_col = work.tile([W, 1], F32)
        nc.scalar.activation(e_col, a2T_ps, mybir.ActivationFunctionType.Exp)
        # softmax denominator -> host divides; keeps the tail short
        S_ps = tl_psp.tile([1, 1], F32, tag="t")
        nc.tensor.matmul(S_ps, lhsT=e_col, rhs=ones_col, start=True, stop=True)
        s_sb = work.tile([1, 1], F32)
        nc.vector.tensor_copy(s_sb, S_ps)
        nc.sync.dma_start(out=sden[:], in_=s_sb)

        att = work.tile([W, F], F32)
        for eng, e2 in ((nc.sync, 0), (nc.scalar, 1)):
            nc.vector.tensor_scalar_mul(
                att[:, e2 * 128 : (e2 + 1) * 128],
                dp_sb[:, e2 * 128 : (e2 + 1) * 128],
                e_col[:],
            )
            eng.dma_start(
                out=y[:, e2 * 128 : (e2 + 1) * 128],
                in_=att[:, e2 * 128 : (e2 + 1) * 128],
            )

    nc.compile()
    return nc


_CACHED = {}


def _get_program():
    if "nc" not in _CACHED:
        _CACHED["nc"] = _build_program()
        _CACHED["kt"] = _host_kt_img()
    return _CACHED["nc"]


def _in_maps(spikes, W1, b1, W2, b2):
    spikes = np.asarray(spikes, dtype=np.float32)
    W1 = np.asarray(W1, dtype=np.float32)
    b1 = np.asarray(b1, dtype=np.float32)
    W2 = np.asarray(W2, dtype=np.float32)
    b2 = np.asarray(b2, dtype=np.float32)
    _get_program()
    # t-major fp16 stream: row r = t*C + c, t = 24w + 4s + p//32, c = p%32
    xall = np.ascontiguousarray(
        spikes.astype(np.float16)
        .reshape(B, C, W, S6, 4, F)
        .transpose(0, 2, 4, 1, 3, 5)
        .reshape(B, W, CH, S6, F)
    )
    # W1 rearranged so chunk m = 2*w + e holds rows d = 256*w + 128*e + p
    w1r = np.ascontiguousarray(
        W1.reshape(W, 2, 128, H).transpose(2, 0, 1, 3).reshape(128, 2 * W * H)
    ).astype(np.float16)
    cimg = _host_cimg(W2, b2)
    cimg[0:H, 80] = b1
    shared = {"ktp": _CACHED["kt"], "w1r": w1r, "cimg": cimg}
    return [{"x": xall[b], **shared} for b in range(B)]


def _assemble(results):
    """Device outputs -> full [B, W*F] fp32 (host applies softmax denom)."""
    out = np.empty((B, W * F), dtype=np.float32)
    for b in range(B):
        yb = np.asarray(results[b]["y"], dtype=np.float32).reshape(W * F)
        sb = float(np.asarray(results[b]["sden"]).reshape(()))
        out[b] = yb / sb
    return out


def kernel(spikes, W1, b1, W2, b2):
    in_maps = _in_maps(spikes, W1, b1, W2, b2)
    res = run_bass_kernel_spmd(_get_program(), in_maps, list(range(B)))
    return _assemble(res.results)


# revision 17
# speedup vs baseline: 1.8164x; 1.1291x over previous
"""Trainium2 Bass kernel for nn_DPSpikingDecoder.

Math: the leaky-integrator scan v_t = 0.5*v_{t-1} + x_t, the channel mean,
and the differential window pooling compose into one linear kernel over
time:  dp[b, w, f] = sum_{c,t} (K[w, t] / C) * spikes[b, c, t, f].
K is banded and window-periodic: window w sees its own 24 steps (kernel
Kd) plus the previous window's 24 steps through the 0.5^d decay tail
(kernel Kt); deeper history is < 1e-7 relative and dropped.

The stream is laid out t-major on host (row r = t*C + c) so each 128-row
chunk is 4 timesteps x 32 channels and 6 chunks complete one window.
The window's PSUM row placement is baked into the STATIONARY: banded
weight images put Kd at column i and Kt at column i+1 of the per-octet
accumulator (i = window mod 8), so eight windows accumulate into one
[33, F] PSUM tile with the A+B combine happening in PSUM — no
cross-partition moves, no SBUF->SBUF DMAs, no per-window engine work.
The last window of each octet parks its tail at partition 32 (a legal
engine-access base; engine ops quantize partition bases to 32), where
the next octet's consumers pick it up.

Each finished octet is staged to SBUF with one wide DVE copy, shipped
to DRAM (output = raw dp rows + octet-boundary tails + softmax row;
host does dp[8o+8] += tail, att = dp * e / sum(e) while unsharding),
and folded into MLP layer 1 (PE transposes + packed matmuls) while the
stream continues — only octet 4 and the tiny softmax chain trail the
last byte.

x is uploaded as float16 (PSUM accumulates fp32): halves HBM traffic,
the roofline for this kernel (fp16 stream measured ~410 GB/s/core).
Rel err ~4e-4 vs the fp32 reference (gate 2e-2).

Sharding: data-parallel over batch B=8 -> one sample per NeuronCore.
"""

import numpy as np
from contextlib import ExitStack

import concourse.bass as bass
import concourse.bacc as bacc
import concourse.tile as tile
from concourse import mybir
from concourse.bass_utils import run_bass_kernel_spmd

F16 = mybir.dt.float16
F32 = mybir.dt.float32

B, C, T, F = 8, 32, 960, 256
L_DP, N_DP = 24, 12
W = T // L_DP            # 40 windows
H = 20                   # hidden dim of the MLP
CH = 128                 # rows per matmul chunk
S6 = 6                   # chunks per window (6 * 128 rows = 24 t * 32 c)
NO = 5                   # octets of 8 windows
# kt image column offsets: G0 band (16/s), G1 octet-opening wide (33/s),
# G2 octet-closing wide (33/s)
G1 = 16 * S6
G2 = G1 + 33 * S6


def _host_K():
    """Exact K[w, t] = differential pooling of the decayed scan."""
    t = np.arange(T)
    d = t[:, None] - t[None, :]
    Lmat = np.where(d >= 0, 0.5 ** np.clip(d, 0, None), 0.0)
    M = np.zeros((W, T))
    for w in range(W):
        M[w, w * L_DP + L_DP - N_DP : w * L_DP + L_DP] = 1.0 / N_DP
        M[w, w * L_DP : w * L_DP + N_DP] -= 1.0 / N_DP
    return M @ Lmat  # [W, T]


def _host_kt_img():
    """[128, 512] fp16 stationary images.  Row p of chunk s has t-offset
    u = 4s + p//32 inside its window.
    G0 band (chunk s at cols 16s..16s+16): Kd at col 7, Kt at col 8 —
      sliced at [7-i, 16-i) it yields a 9-wide stationary with Kd at
      output row i, Kt at row i+1 (octet-relative placement).
    G1 (33 wide): Kd at 0, Kt at 1 — octet's first matmul, start=True
      resets the whole [33, F] accumulator.
    G2 (33 wide): Kd at 7, Kt at 32 — octet's last window parks its
      tail at partition 32 for the next octet's consumers."""
    K = _host_K()
    Kd = K[1, 24:48] / C   # within-window kernel (w-independent, verified)
    Kt = K[2, 24:48] / C   # decay tail onto the next window
    img = np.zeros((CH, 512), dtype=np.float32)
    u = 4 * (np.arange(S6)[None, :]) + (np.arange(CH) // 32)[:, None]  # [128, 6]
    for s in range(S6):
        img[:, 16 * s + 7] = Kd[u[:, s]]
        img[:, 16 * s + 8] = Kt[u[:, s]]
        img[:, G1 + 33 * s + 0] = Kd[u[:, s]]
        img[:, G1 + 33 * s + 1] = Kt[u[:, s]]
        img[:, G2 + 33 * s + 7] = Kd[u[:, s]]
        img[:, G2 + 33 * s + 32] = Kt[u[:, s]]
    return img.astype(np.float16)


def _host_cimg(W2, b2):
    """Packed fp32 consts, one [128, 128] DMA image: cols 0:40 eye(40);
    40:80 [W2; b2]; col 80 b1 (patched in _in_maps); 81:101 the
    4-col-group summing matrix."""
    img = np.zeros((CH, 128), dtype=np.float32)
    img[0:W, 0:W] = np.eye(W, dtype=np.float32)
    img[0:H, 40:80] = W2.astype(np.float32)
    img[H, 40:80] = b2.astype(np.float32)
    for j in range(4):
        for i in range(H):
            img[32 * j + i, 81 + i] = 1.0
    return img


def _build_program():
    nc = bacc.Bacc(None)
    x = nc.declare_dram_parameter("x", [W, CH, S6, F], F16, isOutput=False)
    ktp = nc.declare_dram_parameter("ktp", [CH, 512], F16, isOutput=False)
    w1r = nc.declare_dram_parameter("w1r", [CH, 2 * W * H], F16, isOutput=False)
    cimg = nc.declare_dram_parameter("cimg", [CH, 128], F32, isOutput=False)
    ydp = nc.declare_dram_parameter("ydp", [W, F], F32, isOutput=True)
    yb = nc.declare_dram_parameter("yb", [4, F], F32, isOutput=True)
    es = nc.declare_dram_parameter("es", [1, W + 1], F32, isOutput=True)

    with tile.TileContext(nc) as tc, ExitStack() as ctx:
        consts = ctx.enter_context(tc.tile_pool(name="consts", bufs=1))
        xs = ctx.enter_context(tc.tile_pool(name="xs", bufs=10))
        qp = ctx.enter_context(tc.tile_pool(name="qp", bufs=2))
        tqp = ctx.enter_context(tc.tile_pool(name="tqp", bufs=2))
        dpt = ctx.enter_context(tc.tile_pool(name="dpt", bufs=2))
        work = ctx.enter_context(tc.tile_pool(name="work", bufs=1))
        op_ps = ctx.enter_context(tc.tile_pool(name="op_ps", bufs=2, space="PSUM"))
        tp_psp = ctx.enter_context(tc.tile_pool(name="tp_ps", bufs=1, space="PSUM"))
        tb_psp = ctx.enter_context(tc.tile_pool(name="tb_ps", bufs=2, space="PSUM"))
        hp_psp = ctx.enter_context(tc.tile_pool(name="hp_ps", bufs=1, space="PSUM"))
        tl_psp = ctx.enter_context(tc.tile_pool(name="tl_ps", bufs=2, space="PSUM"))

        kt_sb = consts.tile([CH, 512], F16)
        nc.sync.dma_start(out=kt_sb, in_=ktp[:])
        ci_sb = consts.tile([CH, 128], F32)
        nc.scalar.dma_start(out=ci_sb, in_=cimg[:])
        eye_sb = ci_sb[0:W, 0:W]
        w2b_sb = ci_sb[0 : H + 1, 40:80]
        b1_sb = ci_sb[0:H, 80:81]
        sel_sb = ci_sb[:, 81:101]
        w1_sb = consts.tile([CH, 2 * W * H], F16)

        # augmented MLP input [h; 1] so layer 2 adds b2 inside the matmul
        h_aug = work.tile([H + 1, 1], F32)
        nc.vector.memset(h_aug, 1.0)  # row H stays 1; rows 0..H-1 overwritten
        hp_ps = hp_psp.tile([128, 1], F32)

        def mlp_pair(rhs_col, m2):
            for e in range(2):
                j = m2 % 4
                nc.tensor.matmul(
                    hp_ps[32 * j : 32 * j + H, :],
                    lhsT=w1_sb[:, m2 * H : (m2 + 1) * H],
                    rhs=rhs_col[:, e, :],
                    start=(m2 < 4),
                    stop=(m2 >= 2 * W - 4),
                    tile_position=(0, 32 * j),
                )
                m2 += 1
            return m2

        state = {"m2": 0}
        Qs = {}

        def consume_octet(o):
            """Stage octet o, ship its dp rows, fold into MLP layer 1."""
            Q = qp.tile([33, F], F32, tag="Q", name=f"q{o}")
            nc.vector.tensor_copy(Q, Os[o])
            Qs[o] = Q
            # raw dp rows (+ the parked tail) straight to DRAM on the idle
            # SWDGE ring; host adds the boundary tails and scales
            nc.gpsimd.dma_start(out=ydp[8 * o : 8 * o + 8, :], in_=Q[0:8, :])
            if o < 4:
                nc.gpsimd.dma_start(out=yb[o : o + 1, :], in_=Q[32:33, :])
            tpo = tp_psp.tile([128, 2, 8], F32, tag="tp", name=f"tpo{o}")
            for e in range(2):
                he = slice(128 * e, 128 * (e + 1))
                nc.tensor.transpose(tpo[:, e, :], Q[0:8, he], eye_sb[0:8, 0:8])
            tQ = tqp.tile([128, 2, 8], F32, tag="tQ", name=f"tq{o}")
            nc.vector.tensor_copy(tQ, tpo)
            dpT = dpt.tile([128, 2, 8], F16, tag="dpT", name=f"dpt{o}")
            if o == 0:
                nc.vector.tensor_copy(dpT, tQ)
            else:
                # window 8o's column also needs the previous octet's tail
                tpb = tb_psp.tile([128, 2, 1], F32, tag="tb", name=f"tpb{o}")
                for e in range(2):
                    he = slice(128 * e, 128 * (e + 1))
                    nc.tensor.transpose(
                        tpb[:, e, :], Qs[o - 1][32:33, he],
                        eye_sb[32:33, 32:33],
                    )
                nc.vector.tensor_copy(dpT[:, :, 1:8], tQ[:, :, 1:8])
                nc.vector.tensor_add(dpT[:, :, 0:1], tQ[:, :, 0:1], tpb)
            for i2 in range(8):
                state["m2"] = mlp_pair(dpT[:, :, i2 : i2 + 1], state["m2"])

        Os = {}
        for w in range(W):
            o, i = divmod(w, 8)
            if i == 0:
                Os[o] = op_ps.tile([33, F], F32, tag="O", name=f"o{o}")
            eng = nc.sync if w % 2 == 0 else nc.scalar

            # consume a finished octet two windows later: its staging copy
            # is long done, so the PE never stalls on it
            if w >= 9 and (w - 9) % 8 == 0:
                consume_octet((w - 9) // 8)

            xt = xs.tile([CH, S6, F], F16)
            if w < W - 1:
                eng.dma_start(out=xt, in_=x[w])
            else:
                # last window lands as three 2-chunk slices so its matmuls
                # drain while the final bytes stream in
                for sub, e2 in enumerate((nc.sync, nc.scalar, nc.sync)):
                    e2.dma_start(
                        out=xt[:, 2 * sub : 2 * sub + 2, :],
                        in_=x[w, :, 2 * sub : 2 * sub + 2, :],
                    )
            # w1 rides early on both rings, behind the first x tiles
            if w == 2:
                nc.sync.dma_start(out=w1_sb[:, 0 : W * H], in_=w1r[:, 0 : W * H])
            if w == 3:
                nc.scalar.dma_start(out=w1_sb[:, W * H :], in_=w1r[:, W * H :])

            # ---- window contraction, placement baked into the stationary:
            # Kd -> octet row i, Kt -> row i+1 (row 32 when i == 7)
            for s in range(S6):
                if i == 0 and s == 0:
                    lhsT = kt_sb[:, G1 + 33 * s : G1 + 33 * s + 33]
                    region, start = 33, True
                elif i == 7:
                    lhsT = kt_sb[:, G2 + 33 * s : G2 + 33 * s + 33]
                    region, start = 33, False
                else:
                    lhsT = kt_sb[:, 16 * s + 7 - i : 16 * s + 16 - i]
                    region, start = 9, False
                nc.tensor.matmul(
                    Os[o][0:region, :],
                    lhsT=lhsT,
                    rhs=xt[:, s, :],
                    start=start,
                    stop=(i == 7 and s == S6 - 1),
                )

        consume_octet(4)

        # ---- tail: fold col groups, relu, layer 2, softmax numerators ----
        hp_sb = work.tile([128, 1], F32)
        nc.vector.tensor_copy(hp_sb, hp_ps)
        h_ps = tl_psp.tile([H, 1], F32, tag="t")
        nc.tensor.matmul(h_ps, lhsT=sel_sb, rhs=hp_sb, start=True, stop=True)
        nc.scalar.activation(
            h_aug[0:H, :], h_ps, mybir.ActivationFunctionType.Relu, bias=b1_sb
        )
        a2_ps = tl_psp.tile([1, W], F32, tag="t")
        nc.tensor.matmul(a2_ps, lhsT=h_aug, rhs=w2b_sb, start=True, stop=True)
        es_sb = work.tile([1, W + 1], F32)
        nc.scalar.activation(
            es_sb[0:1, 0:W], a2_ps, mybir.ActivationFunctionType.Exp,
            accum_out=es_sb[0:1, W : W + 1],
        )
        nc.sync.dma_start(out=es[:], in_=es_sb)

    nc.compile()
    return nc


_CACHED = {}


def _get_program():
    if "nc" not in _CACHED:
        _CACHED["nc"] = _build_program()
        _CACHED["kt"] = _host_kt_img()
    return _CACHED["nc"]


def _in_maps(spikes, W1, b1, W2, b2):
    spikes = np.asarray(spikes, dtype=np.float32)
    W1 = np.asarray(W1, dtype=np.float32)
    b1 = np.asarray(b1, dtype=np.float32)
    W2 = np.asarray(W2, dtype=np.float32)
    b2 = np.asarray(b2, dtype=np.float32)
    _get_program()
    # t-major fp16 stream: row r = t*C + c, t = 24w + 4s + p//32, c = p%32
    xall = np.ascontiguousarray(
        spikes.astype(np.float16)
        .reshape(B, C, W, S6, 4, F)
        .transpose(0, 2, 4, 1, 3, 5)
        .reshape(B, W, CH, S6, F)
    )
    # W1 rearranged so chunk m = 2*w + e holds rows d = 256*w + 128*e + p
    w1r = np.ascontiguousarray(
        W1.reshape(W, 2, 128, H).transpose(2, 0, 1, 3).reshape(128, 2 * W * H)
    ).astype(np.float16)
    cimg = _host_cimg(W2, b2)
    cimg[0:H, 80] = b1
    shared = {"ktp": _CACHED["kt"], "w1r": w1r, "cimg": cimg}
    return [{"x": xall[b], **shared} for b in range(B)]


def _assemble(results):
    """Device outputs -> full [B, W*F] fp32.  Host applies the octet
    boundary tails and the softmax scale during unsharding."""
    out = np.empty((B, W * F), dtype=np.float32)
    for b in range(B):
        dp = np.asarray(results[b]["ydp"], dtype=np.float32).reshape(W, F).copy()
        tails = np.asarray(results[b]["yb"], dtype=np.float32).reshape(4, F)
        for o in range(4):
            dp[8 * (o + 1)] += tails[o]
        esv = np.asarray(results[b]["es"], dtype=np.float32).reshape(W + 1)
        ta = esv[0:W] / esv[W]
        out[b] = (dp * ta[:, None]).reshape(W * F)
    return out


def kernel(spikes, W1, b1, W2, b2):
    in_maps = _in_maps(spikes, W1, b1, W2, b2)
    res = run_bass_kernel_spmd(_get_program(), in_maps, list(range(B)))
    return _assemble(res.results)


# revision 19
# speedup vs baseline: 1.8907x; 1.0409x over previous
"""Trainium2 Bass kernel for nn_DPSpikingDecoder.

Math: the leaky-integrator scan v_t = 0.5*v_{t-1} + x_t, the channel mean,
and the differential window pooling compose into one linear kernel over
time:  dp[b, w, f] = sum_{c,t} (K[w, t] / C) * spikes[b, c, t, f].
K is banded and window-periodic: window w sees its own 24 steps (kernel
Kd) plus the previous window's 24 steps through the 0.5^d decay tail
(kernel Kt); deeper history is < 1e-7 relative and dropped.

The stream is laid out t-major on host (row r = t*C + c) so each 128-row
chunk is 4 timesteps x 32 channels and 6 chunks complete one window.
The window's PSUM row placement is baked into the STATIONARY: banded
weight images put Kd at column i and Kt at column i+1 of the per-octet
accumulator (i = window mod 8), so eight windows accumulate into one
[33, F] PSUM tile with the A+B combine happening in PSUM — no
cross-partition moves, no SBUF->SBUF DMAs, no per-window engine work.
The last window of each octet parks its tail at partition 32 (a legal
engine-access base; engine ops quantize partition bases to 32), where
the next octet's consumers pick it up.

Each finished octet is staged to SBUF with one wide DVE copy, shipped
to DRAM (output = raw dp rows + octet-boundary tails + softmax row;
host does dp[8o+8] += tail, att = dp * e / sum(e) while unsharding),
and folded into MLP layer 1 (PE transposes + packed matmuls) while the
stream continues — only octet 4 and the tiny softmax chain trail the
last byte.

x is uploaded as float16 (PSUM accumulates fp32): halves HBM traffic,
the roofline for this kernel (fp16 stream measured ~410 GB/s/core).
Rel err ~4e-4 vs the fp32 reference (gate 2e-2).

Sharding: data-parallel over batch B=8 -> one sample per NeuronCore.
"""

import numpy as np
from contextlib import ExitStack

import concourse.bass as bass
import concourse.bacc as bacc
import concourse.tile as tile
from concourse import mybir
from concourse.bass_utils import run_bass_kernel_spmd

F16 = mybir.dt.float16
F32 = mybir.dt.float32

B, C, T, F = 8, 32, 960, 256
L_DP, N_DP = 24, 12
W = T // L_DP            # 40 windows
H = 20                   # hidden dim of the MLP
CH = 128                 # rows per matmul chunk
S6 = 6                   # chunks per window (6 * 128 rows = 24 t * 32 c)
NO = 5                   # octets of 8 windows
# kt image column offsets: G0 band (16/s), G1 octet-opening wide (33/s),
# G2 octet-closing wide (33/s)
G1 = 16 * S6
G2 = G1 + 33 * S6
G3 = G2 + 33 * S6
G4 = G3 + 33 * S6


def _host_K():
    """Exact K[w, t] = differential pooling of the decayed scan."""
    t = np.arange(T)
    d = t[:, None] - t[None, :]
    Lmat = np.where(d >= 0, 0.5 ** np.clip(d, 0, None), 0.0)
    M = np.zeros((W, T))
    for w in range(W):
        M[w, w * L_DP + L_DP - N_DP : w * L_DP + L_DP] = 1.0 / N_DP
        M[w, w * L_DP : w * L_DP + N_DP] -= 1.0 / N_DP
    return M @ Lmat  # [W, T]


def _host_kt_img():
    """[128, 512] fp16 stationary images.  Row p of chunk s has t-offset
    u = 4s + p//32 inside its window.
    G0 band (chunk s at cols 16s..16s+16): Kd at col 7, Kt at col 8 —
      sliced at [7-i, 16-i) it yields a 9-wide stationary with Kd at
      output row i, Kt at row i+1 (octet-relative placement).
    G1 (33 wide): Kd at 0, Kt at 1 — octet's first matmul, start=True
      resets the whole [33, F] accumulator.
    G2 (33 wide): Kd at 7, Kt at 32 — octet's last window parks its
      tail at partition 32 for the next octet's consumers."""
    K = _host_K()
    Kd = K[1, 24:48] / C   # within-window kernel (w-independent, verified)
    Kt = K[2, 24:48] / C   # decay tail onto the next window
    img = np.zeros((CH, 896), dtype=np.float32)
    u = 4 * (np.arange(S6)[None, :]) + (np.arange(CH) // 32)[:, None]  # [128, 6]
    for s in range(S6):
        img[:, 16 * s + 7] = Kd[u[:, s]]
        img[:, 16 * s + 8] = Kt[u[:, s]]
        img[:, G1 + 33 * s + 0] = Kd[u[:, s]]
        img[:, G1 + 33 * s + 1] = Kt[u[:, s]]
        img[:, G2 + 33 * s + 7] = Kd[u[:, s]]
        img[:, G2 + 33 * s + 32] = Kt[u[:, s]]
        img[:, G3 + 33 * s + 6] = Kd[u[:, s]]
        img[:, G3 + 33 * s + 32] = Kt[u[:, s]]
        img[:, G4 + 33 * s + 0] = Kd[u[:, s]]
        img[:, G4 + 33 * s + 32] = Kt[u[:, s]]
    return img.astype(np.float16)


def _host_cimg(W2, b2):
    """Packed fp32 consts, one [128, 128] DMA image: cols 0:40 eye(40);
    40:80 [W2; b2]; col 80 b1 (patched in _in_maps); 81:101 the
    4-col-group summing matrix."""
    img = np.zeros((CH, 128), dtype=np.float32)
    img[0:W, 0:W] = np.eye(W, dtype=np.float32)
    img[0:H, 40:80] = W2.astype(np.float32)
    img[H, 40:80] = b2.astype(np.float32)
    for j in range(4):
        for i in range(H):
            img[32 * j + i, 81 + i] = 1.0
    return img


def _build_program():
    nc = bacc.Bacc(None)
    x = nc.declare_dram_parameter("x", [W, CH, S6, F], F16, isOutput=False)
    ktp = nc.declare_dram_parameter("ktp", [CH, 896], F16, isOutput=False)
    w1r = nc.declare_dram_parameter("w1r", [CH, 2 * W * H], F16, isOutput=False)
    cimg = nc.declare_dram_parameter("cimg", [CH, 128], F32, isOutput=False)
    ydp = nc.declare_dram_parameter("ydp", [W, F], F32, isOutput=True)
    yb = nc.declare_dram_parameter("yb", [5, F], F32, isOutput=True)
    es = nc.declare_dram_parameter("es", [1, W + 1], F32, isOutput=True)

    with tile.TileContext(nc) as tc, ExitStack() as ctx:
        consts = ctx.enter_context(tc.tile_pool(name="consts", bufs=1))
        xs = ctx.enter_context(tc.tile_pool(name="xs", bufs=10))
        qp = ctx.enter_context(tc.tile_pool(name="qp", bufs=2))
        tqp = ctx.enter_context(tc.tile_pool(name="tqp", bufs=2))
        dpt = ctx.enter_context(tc.tile_pool(name="dpt", bufs=2))
        work = ctx.enter_context(tc.tile_pool(name="work", bufs=1))
        op_ps = ctx.enter_context(tc.tile_pool(name="op_ps", bufs=2, space="PSUM"))
        tp_psp = ctx.enter_context(tc.tile_pool(name="tp_ps", bufs=1, space="PSUM"))
        tb_psp = ctx.enter_context(tc.tile_pool(name="tb_ps", bufs=2, space="PSUM"))
        hp_psp = ctx.enter_context(tc.tile_pool(name="hp_ps", bufs=1, space="PSUM"))
        tl_psp = ctx.enter_context(tc.tile_pool(name="tl_ps", bufs=2, space="PSUM"))

        kt_sb = consts.tile([CH, 896], F16)
        nc.sync.dma_start(out=kt_sb, in_=ktp[:])
        ci_sb = consts.tile([CH, 128], F32)
        nc.scalar.dma_start(out=ci_sb, in_=cimg[:])
        eye_sb = ci_sb[0:W, 0:W]
        w2b_sb = ci_sb[0 : H + 1, 40:80]
        b1_sb = ci_sb[0:H, 80:81]
        sel_sb = ci_sb[:, 81:101]
        w1_sb = consts.tile([CH, 2 * W * H], F16)

        # augmented MLP input [h; 1] so layer 2 adds b2 inside the matmul
        h_aug = work.tile([H + 1, 1], F32)
        nc.vector.memset(h_aug, 1.0)  # row H stays 1; rows 0..H-1 overwritten
        hp_ps = hp_psp.tile([128, 1], F32)

        def mlp_pair(rhs_col, m2):
            for e in range(2):
                j = m2 % 4
                nc.tensor.matmul(
                    hp_ps[32 * j : 32 * j + H, :],
                    lhsT=w1_sb[:, m2 * H : (m2 + 1) * H],
                    rhs=rhs_col[:, e, :],
                    start=(m2 < 4),
                    stop=(m2 >= 2 * W - 4),
                    tile_position=(0, 32 * j),
                )
                m2 += 1
            return m2

        state = {"m2": 0}
        Qs = {}

        def consume_octet(o, bs=8, w0=None):
            """Stage group o (bs windows), ship dp rows, fold into layer 1."""
            w0 = 8 * o if w0 is None else w0
            Q = qp.tile([33, F], F32, tag="Q", name=f"q{o}")
            nc.vector.tensor_copy(Q, Os[o])
            Qs[o] = Q
            # raw dp rows (+ the parked tail) straight to DRAM on the idle
            # SWDGE ring; host adds the boundary tails and scales
            nc.gpsimd.dma_start(out=ydp[w0 : w0 + bs, :], in_=Q[0:bs, :])
            if o < 5:
                nc.gpsimd.dma_start(out=yb[o : o + 1, :], in_=Q[32:33, :])
            tpo = tp_psp.tile([128, 2, 8], F32, tag="tp", name=f"tpo{o}")
            for e in range(2):
                he = slice(128 * e, 128 * (e + 1))
                nc.tensor.transpose(tpo[:, e, 0:bs], Q[0:bs, he],
                                    eye_sb[0:bs, 0:bs])
            tQ = tqp.tile([128, 2, 8], F32, tag="tQ", name=f"tq{o}")
            nc.vector.tensor_copy(tQ[:, :, 0:bs], tpo[:, :, 0:bs])
            dpT = dpt.tile([128, 2, 8], F16, tag="dpT", name=f"dpt{o}")
            if o == 0:
                nc.vector.tensor_copy(dpT, tQ)
            else:
                # the group's first column also needs the previous tail
                tpb = tb_psp.tile([128, 2, 1], F32, tag="tb", name=f"tpb{o}")
                for e in range(2):
                    he = slice(128 * e, 128 * (e + 1))
                    nc.tensor.transpose(
                        tpb[:, e, :], Qs[o - 1][32:33, he],
                        eye_sb[32:33, 32:33],
                    )
                if bs > 1:
                    nc.vector.tensor_copy(dpT[:, :, 1:bs], tQ[:, :, 1:bs])
                nc.vector.tensor_add(dpT[:, :, 0:1], tQ[:, :, 0:1], tpb)
            for i2 in range(bs):
                state["m2"] = mlp_pair(dpT[:, :, i2 : i2 + 1], state["m2"])

        Os = {}
        for w in range(W):
            o, i = divmod(w, 8)
            if w == 39:
                o, i = 5, 0
            if i == 0:
                Os[o] = op_ps.tile([33, F], F32, tag="O", name=f"o{o}")
            eng = nc.sync if w % 2 == 0 else nc.scalar

            # consume a finished group two windows later (group 4 closes
            # at w38 and is consumed while window 39 streams): the staging
            # copy is long done, so the PE never stalls on it
            if w >= 9 and (w - 9) % 8 == 0:
                consume_octet((w - 9) // 8)
            if w == 39:
                consume_octet(4, bs=7)

            xt = xs.tile([CH, S6, F], F16)
            if w < W - 1:
                eng.dma_start(out=xt, in_=x[w])
            else:
                # last window lands as three 2-chunk slices so its matmuls
                # drain while the final bytes stream in
                for sub, e2 in enumerate((nc.sync, nc.scalar, nc.sync)):
                    e2.dma_start(
                        out=xt[:, 2 * sub : 2 * sub + 2, :],
                        in_=x[w, :, 2 * sub : 2 * sub + 2, :],
                    )
            # w1 rides early on both rings, behind the first x tiles
            if w == 2:
                nc.sync.dma_start(out=w1_sb[:, 0 : W * H], in_=w1r[:, 0 : W * H])
            if w == 3:
                nc.scalar.dma_start(out=w1_sb[:, W * H :], in_=w1r[:, W * H :])

            # ---- window contraction, placement baked into the stationary:
            # Kd -> group row i, Kt -> row i+1 (parked at 32 when closing)
            for s in range(S6):
                stop = False
                if w == 39:
                    lhsT = kt_sb[:, G4 + 33 * s : G4 + 33 * s + 33]
                    region, start, stop = 33, (s == 0), (s == S6 - 1)
                elif i == 0 and s == 0:
                    lhsT = kt_sb[:, G1 + 33 * s : G1 + 33 * s + 33]
                    region, start = 33, True
                elif i == 7:
                    lhsT = kt_sb[:, G2 + 33 * s : G2 + 33 * s + 33]
                    region, start = 33, False
                    stop = s == S6 - 1
                elif o == 4 and i == 6:
                    lhsT = kt_sb[:, G3 + 33 * s : G3 + 33 * s + 33]
                    region, start = 33, False
                    stop = s == S6 - 1
                else:
                    lhsT = kt_sb[:, 16 * s + 7 - i : 16 * s + 16 - i]
                    region, start = 9, False
                nc.tensor.matmul(
                    Os[o][0:region, :],
                    lhsT=lhsT,
                    rhs=xt[:, s, :],
                    start=start,
                    stop=stop,
                )

        consume_octet(5, bs=1, w0=39)

        # ---- tail: fold col groups, relu, layer 2, softmax numerators ----
        hp_sb = work.tile([128, 1], F32)
        nc.vector.tensor_copy(hp_sb, hp_ps)
        h_ps = tl_psp.tile([H, 1], F32, tag="t")
        nc.tensor.matmul(h_ps, lhsT=sel_sb, rhs=hp_sb, start=True, stop=True)
        nc.scalar.activation(
            h_aug[0:H, :], h_ps, mybir.ActivationFunctionType.Relu, bias=b1_sb
        )
        a2_ps = tl_psp.tile([1, W], F32, tag="t")
        nc.tensor.matmul(a2_ps, lhsT=h_aug, rhs=w2b_sb, start=True, stop=True)
        es_sb = work.tile([1, W + 1], F32)
        nc.scalar.activation(
            es_sb[0:1, 0:W], a2_ps, mybir.ActivationFunctionType.Exp,
            accum_out=es_sb[0:1, W : W + 1],
        )
        nc.sync.dma_start(out=es[:], in_=es_sb)

    nc.compile()
    return nc


_CACHED = {}


def _get_program():
    if "nc" not in _CACHED:
        _CACHED["nc"] = _build_program()
        _CACHED["kt"] = _host_kt_img()
    return _CACHED["nc"]


def _in_maps(spikes, W1, b1, W2, b2):
    spikes = np.asarray(spikes, dtype=np.float32)
    W1 = np.asarray(W1, dtype=np.float32)
    b1 = np.asarray(b1, dtype=np.float32)
    W2 = np.asarray(W2, dtype=np.float32)
    b2 = np.asarray(b2, dtype=np.float32)
    _get_program()
    # t-major fp16 stream: row r = t*C + c, t = 24w + 4s + p//32, c = p%32
    xall = np.ascontiguousarray(
        spikes.astype(np.float16)
        .reshape(B, C, W, S6, 4, F)
        .transpose(0, 2, 4, 1, 3, 5)
        .reshape(B, W, CH, S6, F)
    )
    # W1 rearranged so chunk m = 2*w + e holds rows d = 256*w + 128*e + p
    w1r = np.ascontiguousarray(
        W1.reshape(W, 2, 128, H).transpose(2, 0, 1, 3).reshape(128, 2 * W * H)
    ).astype(np.float16)
    cimg = _host_cimg(W2, b2)
    cimg[0:H, 80] = b1
    shared = {"ktp": _CACHED["kt"], "w1r": w1r, "cimg": cimg}
    return [{"x": xall[b], **shared} for b in range(B)]


def _assemble(results):
    """Device outputs -> full [B, W*F] fp32.  Host applies the octet
    boundary tails and the softmax scale during unsharding."""
    out = np.empty((B, W * F), dtype=np.float32)
    for b in range(B):
        dp = np.asarray(results[b]["ydp"], dtype=np.float32).reshape(W, F).copy()
        tails = np.asarray(results[b]["yb"], dtype=np.float32).reshape(5, F)
        for o in range(4):
            dp[8 * (o + 1)] += tails[o]
        dp[39] += tails[4]
        esv = np.asarray(results[b]["es"], dtype=np.float32).reshape(W + 1)
        ta = esv[0:W] / esv[W]
        out[b] = (dp * ta[:, None]).reshape(W * F)
    return out


def kernel(spikes, W1, b1, W2, b2):
    in_maps = _in_maps(spikes, W1, b1, W2, b2)
    res = run_bass_kernel_spmd(_get_program(), in_maps, list(range(B)))
    return _assemble(res.results)
